# revision 12
# baseline (speedup 1.0000x reference)
"""Multi-head attention TRN2 Bass kernel (8 NeuronCores, SPMD).

Problem: B=4, S=1024, E=1024, H=16 heads of dim 64, fp32.
    Q = q @ Wq^T (per head), K, V likewise
    scores = Q K^T / 8 ; P = softmax(scores) ; ctx = P V
    out = concat_heads(ctx) @ Wo^T

Sharding: core c handles batch b = c // 2 and head-group g = c % 2
(8 heads each). Each core computes a partial output projection over its
512 concat features; the host sums the two partials per batch.

Schedule design (v2 — dense-PE): the kernel is PE-bound (~97us of
matmul at full clock vs ~70us of exp on ACT), so the emission order is
built to keep the in-order PE stream dense from first to last cycle:
  - Inputs arrive as many small chunk tensors in exact consumption
    order, spread over three DMA issue queues (sync/scalar/vector), so
    the first score matmul only waits for ~2.5MB (wk+xk+wq pair-0 +
    xq sh-0) instead of the whole 10.5MB.
  - ~22 warmup matmuls on zeros ramp the PE p-state during the DMA
    preamble.
  - Attention groups (sh, p) run p-major; every group's 8 score tiles
    each carry one "fill" slot where deferred PE work is emitted:
    remaining Q/K projections, the V projection (regranularized per
    head-pair-pair x 2 t-tiles so it lands just-in-time for ctx), and
    the output projection.
  - Output projection is split per (s-tile, e-half) into a pairs-0..2
    partial (3 matmuls, drained bf16 to SBUF, runs as fill) and a
    pair-3 final (1 matmul + DVE add), so after the last group's
    normalize only ~4us of PE+DVE+DMA remains.
  - softmax without max-subtraction (scores ~N(0,1): exp is safe);
    denominators ride along in the ctx matmul via ones-columns in the
    V-augmented stationary blocks (M=64->128 padding is free since
    matmul cost is the moving free size).
"""

from contextlib import ExitStack

import ml_dtypes
import numpy as np

import concourse.bacc as bacc
import concourse.mybir as mybir
import concourse.tile as tile
from concourse.bass_utils import run_bass_kernel_spmd

B, S, E, H = 4, 1024, 1024, 16
HD = 64          # head dim
HPC = 8          # heads per core
NPAIR = 4        # head pairs per core
NET = 8          # e-tiles (E / 128)
NTT = 8          # t-tiles (S / 128)
P = 128

F32 = mybir.dt.float32
BF16 = mybir.dt.bfloat16
EXP = mybir.ActivationFunctionType.Exp
SCALE = 1.0 / 8.0  # 1/sqrt(HD)
BF = ml_dtypes.bfloat16

N_WARMUP = 22
DEBUG_DUMP = None


def _emit(nc, tc, ctx, aps):
    xq_c, xk_c, xv_c, wq_c, wk_c, wv_c, wo_c, out = aps

    inp = ctx.enter_context(tc.tile_pool(name="inp", bufs=1))
    const = ctx.enter_context(tc.tile_pool(name="const", bufs=1))
    etp = ctx.enter_context(tc.tile_pool(name="etp", bufs=4))
    obp = ctx.enter_context(tc.tile_pool(name="obp", bufs=3))
    csp = ctx.enter_context(tc.tile_pool(name="csp", bufs=2))
    rcp = ctx.enter_context(tc.tile_pool(name="rcp", bufs=2))
    pp_mm = ctx.enter_context(tc.tile_pool(name="pp_mm", bufs=2, space="PSUM"))
    pp_sc = ctx.enter_context(tc.tile_pool(name="pp_sc", bufs=2, space="PSUM"))
    pp_ctx = ctx.enter_context(tc.tile_pool(name="pp_ctx", bufs=2, space="PSUM"))

    # ---- SBUF-resident tiles ----
    qt = const.tile([P, 4096], BF16, name="qt")
    kt = const.tile([P, 4096], BF16, name="kt")
    vaug = const.tile([P, 8192], BF16, name="vaug")
    cat = const.tile([P, 4096], BF16, name="cat")
    opart = const.tile([P, 8192], BF16, name="opart")
    zw = const.tile([P, P], BF16, name="zw")
    zr = const.tile([P, 512], BF16, name="zr")

    # input chunk tiles (filled by one DMA each)
    xq = [inp.tile([P, 2048], BF16, name=f"xq{i}") for i in range(4)]
    xk = [inp.tile([P, 2048], BF16, name=f"xk{i}") for i in range(4)]
    xv = [inp.tile([P, 4096], BF16, name=f"xv{i}") for i in range(2)]
    wq = [inp.tile([P, 2048], BF16, name=f"wq{i}") for i in range(2)]
    wk = [inp.tile([P, 2048], BF16, name=f"wk{i}") for i in range(2)]
    wv = inp.tile([P, 4096], BF16, name="wv")
    wo = [inp.tile([P, 2048], BF16, name=f"wo{i}") for i in range(2)]

    # ---- constants: zeros for warmup, ones blocks of the V augmentation
    # (even heads [V|ones] -> denom rows 64:128; odd heads [ones|V] ->
    # denom rows 0:64) ----
    nc.gpsimd.memset(zw[:], 0.0)
    nc.gpsimd.memset(zr[:], 0.0)
    v4 = vaug[:, :].rearrange("p (j q c) -> p j q c", q=2, c=P)
    nc.gpsimd.memset(v4[:, :, 0, HD:P], 1.0)
    nc.gpsimd.memset(v4[:, :, 1, 0:HD], 1.0)

    # ---- DMA issue, consumption order, three queues ----
    for i in range(4):                      # xk sh0 halves, sh1 halves
        nc.sync.dma_start(out=xk[i][:], in_=xk_c[i][:])
    for i in range(2):                      # xv halves (tt 0-3, 4-7)
        nc.sync.dma_start(out=xv[i][:], in_=xv_c[i][:])
    nc.scalar.dma_start(out=wk[0][:], in_=wk_c[0][:])
    nc.scalar.dma_start(out=wq[0][:], in_=wq_c[0][:])
    for i in range(4):                      # xq sh0 halves, sh1 halves
        nc.scalar.dma_start(out=xq[i][:], in_=xq_c[i][:])
    nc.gpsimd.dma_start(out=wv[:], in_=wv_c[:])
    nc.gpsimd.dma_start(out=wk[1][:], in_=wk_c[1][:])
    nc.gpsimd.dma_start(out=wq[1][:], in_=wq_c[1][:])
    nc.gpsimd.dma_start(out=wo[0][:], in_=wo_c[0][:])
    nc.gpsimd.dma_start(out=wo[1][:], in_=wo_c[1][:])

    # ---- PE warmup during the DMA preamble ----
    for _ in range(N_WARMUP):
        ps = pp_mm.tile([P, 512], F32, name="wu", tag="mm")
        nc.tensor.matmul(ps[:], lhsT=zw[:], rhs=zr[:], start=True, stop=True)

    # ---- PE work units ----
    def proj_half(dst, w, x, p, sh):
        """One [128,512] projection tile: dst pair p, seq half sh."""
        ps = pp_mm.tile([P, 512], F32, name="ps", tag="mm")
        for et in range(NET):
            nc.tensor.matmul(
                ps[:],
                lhsT=w[p // 2][:, (p & 1) * 1024 + et * P:(p & 1) * 1024 + (et + 1) * P],
                rhs=x[sh * 2 + et // 4][:, (et & 3) * 512:((et & 3) + 1) * 512],
                start=(et == 0), stop=(et == NET - 1),
            )
        nc.vector.tensor_copy(dst[:, p * 1024 + sh * 512:p * 1024 + (sh + 1) * 512], ps[:])

    def vproj_unit(tt):
        """V projection for t-tile tt, all 8 heads: one accumulation
        group (8 matmuls of N=512) per psum bank."""
        ps = pp_mm.tile([P, 512], F32, name="psv", tag="mm")
        half, tl = tt // 4, tt % 4
        for et in range(NET):
            nc.tensor.matmul(
                ps[:],
                lhsT=xv[half][:, tl * 1024 + et * P:tl * 1024 + (et + 1) * P],
                rhs=wv[:, et * 512:(et + 1) * 512],
                start=(et == 0), stop=(et == NET - 1),
            )
        dstt = vaug[:, tt * 1024:(tt + 1) * 1024].rearrange(
            "p (j q c) -> p j q c", q=2, c=P)
        srcv = ps[:].rearrange("p (j q c) -> p j q c", q=2, c=HD)
        nc.vector.tensor_copy(dstt[:, :, 0, 0:HD], srcv[:, :, 0, :])
        nc.vector.tensor_copy(dstt[:, :, 1, HD:P], srcv[:, :, 1, :])

    def op_partial(sh, stl, ih):
        """Output projection partial over pairs 0..2 for s-tile
        sh*4+stl, e-half ih; drained bf16 to opart."""
        st = sh * 4 + stl
        ps = pp_mm.tile([P, 512], F32, name="po", tag="mm")
        for p4 in range(3):
            nc.tensor.matmul(
                ps[:],
                lhsT=cat[:, p4 * 1024 + st * P:p4 * 1024 + (st + 1) * P],
                rhs=wo[p4 // 2][:, (p4 & 1) * 1024 + ih * 512:(p4 & 1) * 1024 + (ih + 1) * 512],
                start=(p4 == 0), stop=(p4 == 2),
            )
        u = sh * 8 + stl * 2 + ih
        nc.vector.tensor_copy(opart[:, u * 512:(u + 1) * 512], ps[:])

    def op_final(sh, stl, ih):
        """Pair-3 contribution + partial add + store."""
        st = sh * 4 + stl
        ps = pp_mm.tile([P, 512], F32, name="pf", tag="mm")
        nc.tensor.matmul(
            ps[:],
            lhsT=cat[:, 3 * 1024 + st * P:3 * 1024 + (st + 1) * P],
            rhs=wo[1][:, 1024 + ih * 512:1024 + (ih + 1) * 512],
            start=True, stop=True,
        )
        u = sh * 8 + stl * 2 + ih
        ob = obp.tile([P, 512], F32, name="ob", tag="ob")
        nc.vector.tensor_add(ob[:], ps[:], opart[:, u * 512:(u + 1) * 512])
        nc.sync.dma_start(
            out=out[st * P:(st + 1) * P, ih * 512:(ih + 1) * 512], in_=ob[:])

    def normalize(ctxA, ctxB, sh, p):
        """Softmax normalize both heads of a pair: ctxA has ctx rows
        0:64 / denom rows 64:128, ctxB mirrored. Copies to SBUF first
        (releases psum fast), reciprocal on DVE, cross-partition moves
        via two small DMAs on the gpsimd queue."""
        qcol = p * 1024 + sh * 512
        cs = csp.tile([P, 1024], F32, name="cs", tag="cs")
        nc.vector.tensor_copy(cs[:, 0:512], ctxA[:])
        nc.vector.tensor_copy(cs[:, 512:1024], ctxB[:])
        rc = rcp.tile([P, 1024], F32, name="rc", tag="rc")
        rc2 = rcp.tile([P, 1024], F32, name="rc2", tag="rc2")
        # head B: denom rows 0:64 -> recip direct, move up for the mul
        nc.vector.reciprocal_approx_fast(rc[0:HD, 512:1024], cs[0:HD, 512:1024])
        nc.gpsimd.dma_start(out=rc2[HD:P, 512:1024], in_=rc[0:HD, 512:1024])
        # head A: denom rows 64:128 -> move down, recip, mul in place
        nc.gpsimd.dma_start(out=rc2[0:HD, 0:512], in_=cs[HD:P, 0:512])
        nc.vector.reciprocal_approx_fast(rc[0:HD, 0:512], rc2[0:HD, 0:512])
        nc.vector.tensor_mul(cat[0:HD, qcol:qcol + 512],
                             cs[0:HD, 0:512], rc[0:HD, 0:512])
        nc.vector.tensor_mul(cat[HD:P, qcol:qcol + 512],
                             cs[HD:P, 512:1024], rc2[HD:P, 512:1024])

    # ---- preamble projections: K pair0 sh0, Q pair0 sh0 ----
    proj_half(kt, wk, xk, 0, 0)
    proj_half(qt, wq, xq, 0, 0)

    # ---- static fill schedule: one entry per (group, slot) ----
    def U(fn, *a):
        return lambda: fn(*a)

    FILL = [
        [[U(vproj_unit, 0), U(proj_half, kt, wk, xk, 0, 1)],
         [U(vproj_unit, 1), U(proj_half, qt, wq, xq, 0, 1)],
         U(vproj_unit, 2), U(vproj_unit, 3), U(vproj_unit, 4),
         U(vproj_unit, 5), U(vproj_unit, 6), U(vproj_unit, 7)],
        [U(proj_half, kt, wk, xk, 1, 0), U(proj_half, kt, wk, xk, 1, 1),
         U(proj_half, qt, wq, xq, 1, 0), U(proj_half, qt, wq, xq, 1, 1),
         None, None, None, None],
        [U(proj_half, kt, wk, xk, 2, 0), U(proj_half, kt, wk, xk, 2, 1),
         U(proj_half, qt, wq, xq, 2, 0), U(proj_half, qt, wq, xq, 2, 1),
         None, None, None, None],
        [U(proj_half, kt, wk, xk, 3, 0), U(proj_half, kt, wk, xk, 3, 1),
         None, None, None, None, None, None],
        [U(proj_half, qt, wq, xq, 3, 0), U(proj_half, qt, wq, xq, 3, 1),
         None, None, None, None, None, None],
        [U(op_partial, 0, 0, 0), U(op_partial, 0, 0, 1),
         U(op_partial, 0, 1, 0), U(op_partial, 0, 1, 1),
         U(op_partial, 0, 2, 0), U(op_partial, 0, 2, 1),
         U(op_partial, 0, 3, 0), U(op_partial, 0, 3, 1)],
        [U(op_partial, 1, 0, 0), U(op_partial, 1, 0, 1),
         U(op_partial, 1, 1, 0), U(op_partial, 1, 1, 1),
         U(op_partial, 1, 2, 0), U(op_partial, 1, 2, 1),
         U(op_partial, 1, 3, 0), U(op_partial, 1, 3, 1)],
        [None, None, None, None,
         [U(op_final, 0, 0, 0), U(op_final, 0, 0, 1)],
         [U(op_final, 0, 1, 0), U(op_final, 0, 1, 1)],
         [U(op_final, 0, 2, 0), U(op_final, 0, 2, 1)],
         [U(op_final, 0, 3, 0), U(op_final, 0, 3, 1)]],
    ]

    ORDER = [(0, 0), (1, 0), (0, 1), (1, 1), (0, 2), (1, 2), (0, 3), (1, 3)]

    # ---- attention groups with interleaved fill ----
    for gi, (sh, p) in enumerate(ORDER):
        qcol = p * 1024 + sh * 512
        ctxA = pp_ctx.tile([P, 512], F32, name="ctxA", tag="ctx")
        ctxB = pp_ctx.tile([P, 512], F32, name="ctxB", tag="ctx")
        etiles = [None] * NTT

        def ctx_mms(tt):
            bA = (tt * HPC + 2 * p) * P
            eAB = etiles[tt]
            nc.tensor.matmul(ctxA[:], lhsT=vaug[:, bA:bA + P],
                             rhs=eAB[:, 0:512],
                             start=(tt == 0), stop=(tt == NTT - 1))
            nc.tensor.matmul(ctxB[:], lhsT=vaug[:, bA + P:bA + 2 * P],
                             rhs=eAB[:, 512:1024],
                             start=(tt == 0), stop=(tt == NTT - 1))

        for tt in range(NTT):
            kcol = p * 1024 + tt * P
            sAB = pp_sc.tile([P, 1024], F32, name="sAB", tag="sc")
            nc.tensor.matmul(
                sAB[:, 0:512],
                lhsT=kt[0:HD, kcol:kcol + P],
                rhs=qt[0:HD, qcol:qcol + 512],
                start=True, stop=True)
            nc.tensor.matmul(
                sAB[:, 512:1024],
                lhsT=kt[HD:P, kcol:kcol + P],
                rhs=qt[HD:P, qcol:qcol + 512],
                start=True, stop=True)
            eAB = etp.tile([P, 1024], BF16, name="eAB", tag="et")
            nc.scalar.activation(eAB[:], sAB[:], EXP, scale=SCALE)
            etiles[tt] = eAB
            unit = FILL[gi][tt]
            if unit is not None:
                for u in (unit if isinstance(unit, list) else [unit]):
                    u()
            if tt > 0:
                ctx_mms(tt - 1)
        ctx_mms(NTT - 1)
        normalize(ctxA, ctxB, sh, p)

    # ---- tail: output projection finals for sh=1 ----
    for stl in range(4):
        for ih in range(2):
            op_final(1, stl, ih)

    if DEBUG_DUMP:
        dbg_qt, dbg_kt, dbg_vaug, dbg_cat = DEBUG_DUMP
        nc.sync.dma_start(out=dbg_qt[:], in_=qt[:])
        nc.sync.dma_start(out=dbg_kt[:], in_=kt[:])
        nc.sync.dma_start(out=dbg_vaug[:], in_=vaug[:])
        nc.sync.dma_start(out=dbg_cat[:], in_=cat[:])


_CACHE = {}


def build():
    if "nc" in _CACHE:
        return _CACHE["nc"]
    nc = bacc.Bacc("TRN2", target_bir_lowering=False, debug=False)

    def dr(name, shape):
        return nc.dram_tensor(name, shape, BF16, kind="ExternalInput").ap()

    xq_c = [dr(f"xq{i}", [P, 2048]) for i in range(4)]
    xk_c = [dr(f"xk{i}", [P, 2048]) for i in range(4)]
    xv_c = [dr(f"xv{i}", [P, 4096]) for i in range(2)]
    wq_c = [dr(f"wq{i}", [P, 2048]) for i in range(2)]
    wk_c = [dr(f"wk{i}", [P, 2048]) for i in range(2)]
    wv_c = dr("wv", [P, 4096])
    wo_c = [dr(f"wo{i}", [P, 2048]) for i in range(2)]
    out = nc.dram_tensor("out", [S, E], F32, kind="ExternalOutput").ap()
    global DEBUG_DUMP
    if _CACHE.get("debug"):
        DEBUG_DUMP = (
            nc.dram_tensor("dbg_qt", [P, 4096], BF16, kind="ExternalOutput").ap(),
            nc.dram_tensor("dbg_kt", [P, 4096], BF16, kind="ExternalOutput").ap(),
            nc.dram_tensor("dbg_vaug", [P, 8192], BF16, kind="ExternalOutput").ap(),
            nc.dram_tensor("dbg_cat", [P, 4096], BF16, kind="ExternalOutput").ap(),
        )
    with tile.TileContext(nc) as tc, ExitStack() as ctx:
        _emit(nc, tc, ctx, (xq_c, xk_c, xv_c, wq_c, wk_c, wv_c, wo_c, out))
    nc.compile()
    _CACHE["nc"] = nc
    return nc


def make_in_maps(query, key, value, Wq, Wk, Wv, Wo):
    in_maps = []
    for c in range(8):
        b, g = divmod(c, 2)
        hs = slice(g * HPC, (g + 1) * HPC)
        m = {}

        def bf(a):
            return np.ascontiguousarray(a).astype(BF)

        # x^T [E, S] chunks: (sh, half) -> [128, 4x512]
        for nm, xt in (("xq", query), ("xk", key)):
            xT = np.asarray(xt[b], np.float32).T.reshape(NET, P, S)
            for sh in range(2):
                for half in range(2):
                    chunk = xT[half * 4:(half + 1) * 4, :, sh * 512:(sh + 1) * 512]
                    m[f"{nm}{sh * 2 + half}"] = bf(
                        chunk.transpose(1, 0, 2).reshape(P, 2048))
        # xv: tt-major chunks: [128, 4tt x (8et x 128)]
        xvT = np.asarray(value[b], np.float32).T.reshape(NET, P, NTT, P)
        for half in range(2):
            chunk = xvT[:, :, half * 4:(half + 1) * 4, :]  # [et, r, tl, c]
            m[f"xv{half}"] = bf(chunk.transpose(1, 2, 0, 3).reshape(P, 4096))
        # wq/wk: pair-major chunks [128, 2 x (8et x 128)]
        for nm, wt in (("wq", Wq), ("wk", Wk)):
            wT = np.asarray(wt[hs], np.float32).transpose(2, 0, 1).reshape(
                E, HPC * HD).reshape(NET, P, NPAIR, P)  # [et, r, p, c]
            for half in range(2):
                chunk = wT[:, :, 2 * half:2 * half + 2, :]
                m[f"{nm}{half}"] = bf(chunk.transpose(1, 2, 0, 3).reshape(P, 2048))
        # wv: single chunk, et-major all heads: [128, 8et x 512]
        wvT = np.asarray(Wv[hs], np.float32).transpose(2, 0, 1).reshape(
            E, HPC * HD).reshape(NET, P, 512)  # [et, r, c]
        m["wv"] = bf(wvT.transpose(1, 0, 2).reshape(P, 4096))
        # wo: [512, E] pair chunks [128, 2 x 1024]
        woT = np.asarray(
            Wo[:, g * HPC * HD:(g + 1) * HPC * HD], np.float32).T.reshape(
            NPAIR, P, E)  # [p4, r, i]
        for half in range(2):
            chunk = woT[2 * half:2 * half + 2]  # [2, r, i]
            m[f"wo{half}"] = bf(chunk.transpose(1, 0, 2).reshape(P, 2048))
        in_maps.append(m)
    return in_maps


def kernel(query, key, value, Wq, Wk, Wv, Wo):
    nc = build()
    in_maps = make_in_maps(query, key, value, Wq, Wk, Wv, Wo)
    res = run_bass_kernel_spmd(nc, in_maps, list(range(8))).results
    out = np.empty((B, S, E), np.float32)
    for b in range(B):
        out[b] = res[2 * b]["out"] + res[2 * b + 1]["out"]
    return out


# revision 23
# speedup vs baseline: 1.0647x; 1.0647x over previous
"""Multi-head attention TRN2 Bass kernel (8 NeuronCores, SPMD). Baseline restore."""

from contextlib import ExitStack

import ml_dtypes
import numpy as np

import concourse.bacc as bacc
import concourse.mybir as mybir
import concourse.tile as tile
from concourse.bass_utils import run_bass_kernel_spmd

B, S, E, H = 4, 1024, 1024, 16
HD = 64          # head dim
HPC = 8          # heads per core
NPAIR = 4        # head pairs per core
NET = 8          # e-tiles (E / 128)
NTT = 8          # t-tiles (S / 128)
P = 128

F32 = mybir.dt.float32
BF16 = mybir.dt.bfloat16
EXP = mybir.ActivationFunctionType.Exp
SCALE = 1.0 / 8.0  # 1/sqrt(HD)
BF = ml_dtypes.bfloat16


def _emit(nc, tc, ctx, aps):
    xqT, xkT, xvT, wqT, wkT, wvT, woT, out = aps

    xpool = ctx.enter_context(tc.tile_pool(name="xpool", bufs=3))
    wpool = ctx.enter_context(tc.tile_pool(name="wpool", bufs=3))
    const = ctx.enter_context(tc.tile_pool(name="const", bufs=1))
    etp = ctx.enter_context(tc.tile_pool(name="etp", bufs=16))
    obp = ctx.enter_context(tc.tile_pool(name="obp", bufs=3))
    rcp = ctx.enter_context(tc.tile_pool(name="rcp", bufs=8))
    pp_mm = ctx.enter_context(tc.tile_pool(name="pp_mm", bufs=2, space="PSUM"))
    pp_sc = ctx.enter_context(tc.tile_pool(name="pp_sc", bufs=2, space="PSUM"))
    pp_ctx = ctx.enter_context(tc.tile_pool(name="pp_ctx", bufs=2, space="PSUM"))

    wo_t = const.tile([P, 4096], BF16, name="wo_t")
    qt = const.tile([P, 4096], BF16, name="qt")
    kt = const.tile([P, 4096], BF16, name="kt")
    vaug = const.tile([P, 8192], BF16, name="vaug")
    cat = const.tile([P, 4096], BF16, name="cat")

    # ones blocks of the V augmentation (see module docstring)
    v4 = vaug[:, :].rearrange("p (j q c) -> p j q c", q=2, c=P)
    nc.gpsimd.memset(v4[:, :, 0, HD:P], 1.0)
    nc.gpsimd.memset(v4[:, :, 1, 0:HD], 1.0)

    def load_wx(wT, xT):
        w = wpool.tile([P, NET * 512], BF16, name="w", tag="wt")
        nc.sync.dma_start(out=w[:], in_=wT[:])
        x = xpool.tile([P, NET * 1024], BF16, name="x", tag="xt")
        half = NET * 512
        nc.sync.dma_start(out=x[:, 0:half], in_=xT[:, 0:half])
        nc.sync.dma_start(out=x[:, half:2 * half], in_=xT[:, half:2 * half])
        return w, x

    wq, xq = load_wx(wqT, xqT)
    wk, xk = load_wx(wkT, xkT)
    wv, xv = load_wx(wvT, xvT)
    nc.sync.dma_start(out=wo_t[:], in_=woT[:])

    def proj_pair(w, x, dst, p):
        for sh in range(2):
            ps = pp_mm.tile([P, 512], F32, name="ps", tag="mm")
            for et in range(NET):
                nc.tensor.matmul(
                    ps[:],
                    lhsT=w[:, et * 512 + p * P:et * 512 + (p + 1) * P],
                    rhs=x[:, et * 1024 + sh * 512:et * 1024 + (sh + 1) * 512],
                    start=(et == 0), stop=(et == NET - 1),
                )
            nc.vector.tensor_copy(
                dst[:, p * 1024 + sh * 512:p * 1024 + (sh + 1) * 512], ps[:])

    def normalize_a(ctx_ps, qcol):
        rA = rcp.tile([P, 512], F32, name="rA", tag="rc")
        rA2 = rcp.tile([P, 512], F32, name="rA2", tag="rc")
        nc.vector.tensor_copy(rA[HD:P, :], ctx_ps[HD:P, :])
        nc.sync.dma_start(out=rA[0:HD, :], in_=rA[HD:P, :])
        nc.vector.reciprocal_approx_fast(rA2[0:HD, :], rA[0:HD, :])
        nc.vector.tensor_mul(cat[0:HD, qcol:qcol + 512],
                             ctx_ps[0:HD, :], rA2[0:HD, :])

    def normalize_b(ctx_ps, qcol):
        rB = rcp.tile([P, 512], F32, name="rB", tag="rc")
        nc.vector.reciprocal_approx_fast(rB[0:HD, :], ctx_ps[0:HD, :])
        nc.sync.dma_start(out=rB[HD:P, :], in_=rB[0:HD, :])
        nc.vector.tensor_mul(cat[HD:P, qcol:qcol + 512],
                             ctx_ps[HD:P, :], rB[HD:P, :])

    def attention_pair(sh, p):
            qcol = p * 1024 + sh * 512
            ctxA = pp_ctx.tile([P, 512], F32, name="ctxA", tag="ctx")
            ctxB = pp_ctx.tile([P, 512], F32, name="ctxB", tag="ctx")
            for tt in range(NTT):
                kcol = p * 1024 + tt * P
                sAB = pp_sc.tile([P, 1024], F32, name="sAB", tag="sc")
                nc.tensor.matmul(
                    sAB[:, 0:512],
                    lhsT=kt[0:HD, kcol:kcol + P],
                    rhs=qt[0:HD, qcol:qcol + 512],
                    start=True, stop=True)
                nc.tensor.matmul(
                    sAB[:, 512:1024],
                    lhsT=kt[HD:P, kcol:kcol + P],
                    rhs=qt[HD:P, qcol:qcol + 512],
                    start=True, stop=True)
                eAB = etp.tile([P, 1024], BF16, name="eAB", tag="et")
                nc.scalar.activation(eAB[:], sAB[:], EXP, scale=SCALE)
                bA = (tt * HPC + 2 * p) * P
                bB = bA + P
                nc.tensor.matmul(ctxA[:], lhsT=vaug[:, bA:bA + P],
                                 rhs=eAB[:, 0:512],
                                 start=(tt == 0), stop=(tt == NTT - 1))
                nc.tensor.matmul(ctxB[:], lhsT=vaug[:, bB:bB + P],
                                 rhs=eAB[:, 512:1024],
                                 start=(tt == 0), stop=(tt == NTT - 1))
            normalize_a(ctxA, qcol)
            normalize_b(ctxB, qcol)

    def outproj(sh):
        if sh == 0:
            for j in range(4):
                st = sh * 4 + j
                for ih in range(2):
                    ps = pp_mm.tile([P, 512], F32, name="po", tag="mm")
                    for p4 in range(4):
                        nc.tensor.matmul(
                            ps[:],
                            lhsT=cat[:, p4 * 1024 + st * P:p4 * 1024 + (st + 1) * P],
                            rhs=wo_t[:, p4 * 1024 + ih * 512:p4 * 1024 + (ih + 1) * 512],
                            start=(p4 == 0), stop=(p4 == 3))
                    ob = obp.tile([P, 512], F32, name="ob", tag="ob")
                    nc.vector.tensor_copy(ob[:], ps[:])
                    nc.sync.dma_start(
                        out=out[st * P:(st + 1) * P, ih * 512:(ih + 1) * 512],
                        in_=ob[:])
        else:
            for j in range(4):
                st = sh * 4 + j
                ps = pp_sc.tile([P, 1024], F32, name="po2", tag="sc")
                for k4 in range(4):
                    p4 = (k4 + j) % 4 if j < 2 else k4
                    lhsT = cat[:, p4 * 1024 + st * P:p4 * 1024 + (st + 1) * P]
                    for ih in range(2):
                        nc.tensor.matmul(
                            ps[:, ih * 512:(ih + 1) * 512],
                            lhsT=lhsT,
                            rhs=wo_t[:, p4 * 1024 + ih * 512:p4 * 1024 + (ih + 1) * 512],
                            start=(k4 == 0), stop=(k4 == 3))
                ob = obp.tile([P, 1024], F32, name="ob2", tag="ob2")
                nc.vector.tensor_copy(ob[:], ps[:])
                nc.sync.dma_start(out=out[st * P:(st + 1) * P, :], in_=ob[:])

    def vproj():
      for tt in range(NTT):
        ps = pp_mm.tile([P, 512], F32, name="psv", tag="mm")
        for et in range(NET):
            nc.tensor.matmul(
                ps[:],
                lhsT=xv[:, et * 1024 + tt * P:et * 1024 + (tt + 1) * P],
                rhs=wv[:, et * 512:(et + 1) * 512],
                start=(et == 0), stop=(et == NET - 1),
            )
        dstt = vaug[:, tt * 1024:(tt + 1) * 1024].rearrange(
            "p (j q c) -> p j q c", q=2, c=P)
        srcv = ps[:].rearrange("p (j q c) -> p j q c", q=2, c=HD)
        nc.vector.tensor_copy(dstt[:, :, 0, 0:HD], srcv[:, :, 0, :])
        nc.vector.tensor_copy(dstt[:, :, 1, HD:P], srcv[:, :, 1, :])

    proj_pair(wq, xq, qt, 0)
    proj_pair(wk, xk, kt, 0)
    vproj()
    for p in range(NPAIR):
        if p > 0:
            proj_pair(wq, xq, qt, p)
            proj_pair(wk, xk, kt, p)
        attention_pair(0, p)
        if p == NPAIR - 1:
            outproj(0)
        attention_pair(1, p)
    outproj(1)


_CACHE = {}


def build():
    if "nc" in _CACHE:
        return _CACHE["nc"]
    nc = bacc.Bacc("TRN2", target_bir_lowering=False, debug=False)
    xqT = nc.dram_tensor("xqT", [P, NET * S], BF16, kind="ExternalInput").ap()
    xkT = nc.dram_tensor("xkT", [P, NET * S], BF16, kind="ExternalInput").ap()
    xvT = nc.dram_tensor("xvT", [P, NET * S], BF16, kind="ExternalInput").ap()
    wqT = nc.dram_tensor("wqT", [P, NET * HPC * HD], BF16, kind="ExternalInput").ap()
    wkT = nc.dram_tensor("wkT", [P, NET * HPC * HD], BF16, kind="ExternalInput").ap()
    wvT = nc.dram_tensor("wvT", [P, NET * HPC * HD], BF16, kind="ExternalInput").ap()
    woT = nc.dram_tensor("woT", [P, 4 * E], BF16, kind="ExternalInput").ap()
    out = nc.dram_tensor("out", [S, E], F32, kind="ExternalOutput").ap()
    with tile.TileContext(nc) as tc, ExitStack() as ctx:
        _emit(nc, tc, ctx, (xqT, xkT, xvT, wqT, wkT, wvT, woT, out))
    nc.compile()
    _CACHE["nc"] = nc
    return nc


def make_in_maps(query, key, value, Wq, Wk, Wv, Wo):
    in_maps = []
    for c in range(8):
        b, g = divmod(c, 2)
        hs = slice(g * HPC, (g + 1) * HPC)

        def bf(a):
            return np.ascontiguousarray(a).astype(BF)

        def sbuf_tile(a):
            et = a.shape[0] // P
            return bf(a.reshape(et, P, -1).transpose(1, 0, 2).reshape(P, -1))

        in_maps.append({
            "xqT": sbuf_tile(np.asarray(query[b], np.float32).T),
            "xkT": sbuf_tile(np.asarray(key[b], np.float32).T),
            "xvT": sbuf_tile(np.asarray(value[b], np.float32).T),
            "wqT": sbuf_tile(np.asarray(Wq[hs], np.float32).transpose(2, 0, 1).reshape(E, HPC * HD)),
            "wkT": sbuf_tile(np.asarray(Wk[hs], np.float32).transpose(2, 0, 1).reshape(E, HPC * HD)),
            "wvT": sbuf_tile(np.asarray(Wv[hs], np.float32).transpose(2, 0, 1).reshape(E, HPC * HD)),
            "woT": sbuf_tile(np.asarray(Wo[:, g * HPC * HD:(g + 1) * HPC * HD], np.float32).T),
        })
    return in_maps


def kernel(query, key, value, Wq, Wk, Wv, Wo):
    nc = build()
    in_maps = make_in_maps(query, key, value, Wq, Wk, Wv, Wo)
    res = run_bass_kernel_spmd(nc, in_maps, list(range(8))).results
    out = np.empty((B, S, E), np.float32)
    for b in range(B):
        out[b] = res[2 * b]["out"] + res[2 * b + 1]["out"]
    return out
